# revision 14
# baseline (speedup 1.0000x reference)
"""AttentionResidualGRU fused Trainium2 kernel (v2: pipelined halves).

Strategy: pure data parallelism over batch (8 cores x 32 rows). Both scans
run fused in one 1025-iteration stream.

v2 restructure vs baseline:
- Gates split into feature halves A(0:256)/B(256:512). PE stream reordered
  so the next step's j0/j1 matmuls + k4 run during the current gate chains;
  only the j2/j3 tail is in the serial loop. Keeps PE continuously busy
  (pstate ramp) and shortens the per-step dependency cycle.
- PSUM regions: rzA=[rA|zA], rzB=[rB|zB], C=[CA|CB], D, E (2 bufs),
  misc=[F|xT], Tt (transposes).
- scan2: tensor_tensor_reduce fuses the wd-dot (+bias init),
  scalar_tensor_tensor fuses x' = aw*E01 + (x_prev+E23); x state lives in
  out_buf columns; x^T via PE transpose.
- Engine balance: chain-B (loop-critical) on act+DVE; chain-A z-muls on
  gpsimd; Tt copies split act/DVE.
"""

import os
import sys

import numpy as np
import ml_dtypes

BF16 = ml_dtypes.bfloat16

for _p in ("/opt/trn_rl_repo", "/root/.axon_site/_ro/trn_rl_repo"):
    if os.path.isdir(_p) and _p not in sys.path:
        sys.path.append(_p)

B, T, H, IN, OUT = 256, 1024, 512, 2, 2
NCORES = 8
Bc = B // NCORES          # 32
HH = H // 2               # 256
NE = 4
NCOL_H = 1536 + NE        # h-tile rhs cols: rzA|rzB|CA|CB|E
NCOL_4 = 2048 + NE        # k4 rhs cols:     rzA|rzB|CA|CB|D|E
SV_COLS = Bc * (T + 1)

_PROG_CACHE = {}


# ----------------------------------------------------------------- host prep

def _prep_consts(W_ih, W_hh, b_ih, b_hh, Wa1, ba1, Wa2, ba2, Wr, br):
    f = np.float32
    W_ih = np.asarray(W_ih, f); W_hh = np.asarray(W_hh, f)
    b_ih = np.asarray(b_ih, f); b_hh = np.asarray(b_hh, f)
    Wa1 = np.asarray(Wa1, f); ba1 = np.asarray(ba1, f)
    Wa2 = np.asarray(Wa2, f); ba2 = np.asarray(ba2, f)
    Wr = np.asarray(Wr, f); br = np.asarray(br, f)

    # gate-column order: rA zA | rB zB | CA CB  (r=0:512, z=512:1024,
    # n=1024:1536 in torch layout; A = features 0:256, B = 256:512)
    gcols = np.concatenate([
        np.arange(0, 256), np.arange(512, 768),        # rA zA
        np.arange(256, 512), np.arange(768, 1024),     # rB zB
        np.arange(1024, 1280), np.arange(1280, 1536),  # CA CB
    ])

    wh = np.zeros((512, NCOL_H), f)
    for j in range(4):
        fsl = slice(128 * j, 128 * (j + 1))
        m = wh[fsl]
        m[:, 0:1536] = W_hh[gcols, fsl.start:fsl.stop].T
        m[:, 1536] = Wr[0, fsl]
        m[:, 1537] = -Wr[1, fsl]
        m[:, 1539] = Wr[1, fsl]

    # stationary rows are ordered [1, v0_prev, v1_prev, v0, v1]
    w4 = np.zeros((5, NCOL_4), f)
    w4[3:5, 0:1536] = W_ih[gcols, :].T
    bsum = b_ih + b_hh
    w4[0, 0:1024] = bsum[gcols[0:1024]]
    w4[0, 1024:1536] = b_hh[gcols[1024:1536]]
    w4[3:5, 1536:2048] = W_ih[1024:1536, :].T
    w4[0, 1536:2048] = b_ih[1024:1536]
    w4[0, 2048] = br[0]
    w4[0, 2049] = -br[1]
    w4[1, 2050] = 1.0
    w4[2, 2051] = 1.0
    w4[0, 2051] = br[1]

    # attention MLP: fold |wd| (wd = Wa2[0]-Wa2[1]) into Wa1/ba1 and
    # reorder hidden units so wd>=0 units come first. Then
    # dt = sum(relu_pos) - sum(relu_neg), computed via act accum_out.
    wd = (Wa2[0] - Wa2[1]).astype(f)
    perm = np.argsort(wd < 0, kind="stable")
    n_pos = int((wd >= 0).sum())
    awd = np.abs(wd[perm])
    Wa1p = Wa1[perm] * awd[:, None]
    ba1p = ba1[perm] * awd
    wax = Wa1p[:, 0:2].T.copy()   # [2, HH]
    wav = np.zeros((3, HH), f)    # rows [1, v0p, v1p]
    wav[0] = ba1p
    wav[1] = Wa1p[:, 2]
    wav[2] = Wa1p[:, 3]

    wdb = np.broadcast_to(wd, (Bc, HH)).astype(BF16)
    dbias = np.full((Bc, 1), float(ba2[0] - ba2[1]), f)
    ident = np.eye(32, dtype=f).astype(BF16)
    identf = np.eye(32, dtype=f)
    return dict(wh=wh.astype(BF16), w4=w4.astype(BF16), wax=wax.astype(BF16),
                wav=wav.astype(BF16), wdb=wdb, dbias=dbias, ident=ident,
                identf=identf), n_pos


def _prep_core(c, X0, V):
    f = np.float32
    bs = slice(Bc * c, Bc * (c + 1))
    Vc = np.asarray(V[bs], f)                      # [32, T, 2]
    sv5 = np.zeros((5, SV_COLS), f)    # rows [1, v0p, v1p, v0, v1]
    v0 = Vc[:, :, 0].T.reshape(-1)                 # [T*32] time-major
    v1 = Vc[:, :, 1].T.reshape(-1)
    sv5[0, :] = 1.0
    sv5[1, Bc:] = v0
    sv5[2, Bc:] = v1
    sv5[3, : Bc * T] = v0
    sv5[4, : Bc * T] = v1
    x0 = np.asarray(X0[bs], f)                     # [32, 2]
    x0t = np.zeros((Bc, 32), f)
    x0t[0:2, :] = x0.T                             # xtb init (rows 0-1)
    return dict(sv5=sv5.astype(BF16), x0=x0, x0t=x0t.astype(BF16))


# ------------------------------------------------------------- device program

def _build_program(t_steps, n_pos):
    from concourse import bacc, tile, mybir  # noqa

    f32 = mybir.dt.float32
    bf16 = mybir.dt.bfloat16
    AF = mybir.ActivationFunctionType
    OP = mybir.AluOpType

    nc = bacc.Bacc(None)
    d_wh = nc.declare_dram_parameter("wh", [512, NCOL_H], bf16, isOutput=False)
    d_w4 = nc.declare_dram_parameter("w4", [5, NCOL_4], bf16, isOutput=False)
    d_wax = nc.declare_dram_parameter("wax", [2, HH], bf16, isOutput=False)
    d_wav = nc.declare_dram_parameter("wav", [3, HH], bf16, isOutput=False)
    d_wdb = nc.declare_dram_parameter("wdb", [Bc, HH], bf16, isOutput=False)
    d_dbias = nc.declare_dram_parameter("dbias", [Bc, 1], f32, isOutput=False)
    d_ident = nc.declare_dram_parameter("ident", [32, 32], bf16, isOutput=False)
    d_identf = nc.declare_dram_parameter("identf", [32, 32], f32,
                                         isOutput=False)
    d_sv5 = nc.declare_dram_parameter("sv5", [5, SV_COLS], bf16, isOutput=False)
    d_x0 = nc.declare_dram_parameter("x0", [Bc, 2], f32, isOutput=False)
    d_x0t = nc.declare_dram_parameter("x0t", [Bc, 32], bf16, isOutput=False)
    d_out = nc.declare_dram_parameter("out", [Bc, 2 * T], f32, isOutput=True)

    with tile.TileContext(nc) as tc:
        with (
            tc.tile_pool(name="const", bufs=1) as cpool,
            tc.tile_pool(name="state", bufs=1) as spool,
            tc.tile_pool(name="work", bufs=2) as wpool,
            tc.tile_pool(name="p_rza", bufs=1, space="PSUM") as p_rza,
            tc.tile_pool(name="p_rzb", bufs=1, space="PSUM") as p_rzb,
            tc.tile_pool(name="p_c", bufs=1, space="PSUM") as p_c,
            tc.tile_pool(name="p_d", bufs=1, space="PSUM") as p_d,
            tc.tile_pool(name="p_e", bufs=2, space="PSUM") as p_e,
            tc.tile_pool(name="p_t", bufs=1, space="PSUM") as p_t,
            tc.tile_pool(name="p_m", bufs=1, space="PSUM") as p_m,
        ):
            # constants
            wh_t = [cpool.tile([128, NCOL_H], bf16, tag=f"wh{j}",
                               name=f"wh{j}")
                    for j in range(4)]
            w4_t = cpool.tile([5, NCOL_4], bf16, tag="w4")
            wax_t = cpool.tile([2, HH], bf16, tag="wax")
            wav_t = cpool.tile([3, HH], bf16, tag="wav")
            wdb_t = cpool.tile([Bc, HH], bf16, tag="wdb")
            dbias_t = cpool.tile([Bc, 1], f32, tag="dbias")
            id_t = cpool.tile([32, 32], bf16, tag="ident")
            idf_t = cpool.tile([32, 32], f32, tag="identf")
            sv8 = cpool.tile([35, SV_COLS], bf16, tag="sv8")
            for j in range(4):
                nc.sync.dma_start(out=wh_t[j][:],
                                  in_=d_wh[128 * j:128 * (j + 1), :])
            nc.sync.dma_start(out=w4_t[:], in_=d_w4[:])
            nc.sync.dma_start(out=wax_t[:], in_=d_wax[:])
            nc.sync.dma_start(out=wav_t[:], in_=d_wav[:])
            nc.sync.dma_start(out=wdb_t[:], in_=d_wdb[:])
            nc.sync.dma_start(out=dbias_t[:], in_=d_dbias[:])
            nc.sync.dma_start(out=id_t[:], in_=d_ident[:])
            nc.sync.dma_start(out=idf_t[:], in_=d_identf[:])
            nc.sync.dma_start(out=sv8[0:5, :], in_=d_sv5[:])

            # state
            h_tiles = spool.tile([128, 128], bf16, tag="h_tiles")
            h_bm = spool.tile([Bc, H], bf16, tag="h_bm")
            rz_sb = spool.tile([Bc, 1024], bf16, tag="rz_sb")
            n_sb = spool.tile([Bc, H], bf16, tag="n_sb")
            xtb = spool.tile([Bc, 32], bf16, tag="xtb")
            xtf = spool.tile([Bc, 32], f32, tag="xtf")
            xstg = spool.tile([Bc, 32], f32, tag="xstg")
            x0_t = spool.tile([Bc, 2], f32, tag="x0")
            out_buf = spool.tile([Bc, 2 * T + 32], f32, tag="out_buf")
            nc.vector.memset(h_bm[:], 0.0)
            nc.vector.memset(xstg[:], 0.0)
            nc.sync.dma_start(out=x0_t[:], in_=d_x0[:])
            nc.sync.dma_start(out=xtb[:], in_=d_x0t[:])

            mm = nc.tensor.matmul

            # persistent PSUM tiles (managed manually across iterations)
            rzA = p_rza.tile([Bc, 512], f32, tag="rzA")
            rzB = p_rzb.tile([Bc, 512], f32, tag="rzB")
            C = p_c.tile([Bc, 512], f32, tag="C")
            D = p_d.tile([Bc, 512], f32, tag="D")
            Tt = p_t.tile([128, 128], bf16, tag="Tt")
            misc = p_m.tile([Bc, 320], f32, tag="misc")
            F = misc[:, 0:HH]

            def k4(s, first=False):
                """gx matmuls for step s (start all PSUM groups)."""
                sl4 = sv8[0:5, Bc * s: Bc * (s + 1)]
                st = first
                E = p_e.tile([Bc, NE], f32, tag="E")
                mm(E[:], sl4, w4_t[:, 2048:2052], start=True, stop=first)
                if s < t_steps:
                    mm(rzA[:], sl4, w4_t[:, 0:512], start=True, stop=st)
                    # C halves share one PSUM bank (= one accumulation
                    # group/zero region): only CA starts, only CB stops.
                    mm(C[:, 0:256], sl4, w4_t[:, 1024:1280],
                       start=True, stop=False)
                    mm(rzB[:], sl4, w4_t[:, 512:1024], start=True, stop=st)
                    mm(C[:, 256:512], sl4, w4_t[:, 1280:1536],
                       start=False, stop=st)
                    mm(D[:], sl4, w4_t[:, 1536:2048], start=True, stop=True)
                return E

            def stat(j):
                return h_tiles[:, 32 * j: 32 * (j + 1)]

            def tail_mms(s, E):
                """j2/j3 h-matmuls; A group first; stops on j3."""
                if s < t_steps:
                    mm(rzA[:], stat(2), wh_t[2][:, 0:512],
                       start=False, stop=False)
                    mm(rzA[:], stat(3), wh_t[3][:, 0:512],
                       start=False, stop=True)
                    mm(C[:, 0:256], stat(2), wh_t[2][:, 1024:1280],
                       start=False, stop=False)
                    mm(C[:, 0:256], stat(3), wh_t[3][:, 1024:1280],
                       start=False, stop=False)
                    mm(rzB[:], stat(2), wh_t[2][:, 512:1024],
                       start=False, stop=False)
                    mm(rzB[:], stat(3), wh_t[3][:, 512:1024],
                       start=False, stop=True)
                    mm(C[:, 256:512], stat(2), wh_t[2][:, 1280:1536],
                       start=False, stop=False)
                    mm(C[:, 256:512], stat(3), wh_t[3][:, 1280:1536],
                       start=False, stop=True)
                mm(E[:], stat(2), wh_t[2][:, 1536:1540],
                   start=False, stop=False)
                mm(E[:], stat(3), wh_t[3][:, 1536:1540],
                   start=False, stop=True)

            def early_mms(s, E):
                """j0/j1 h-matmuls for step s (no stops)."""
                if s < t_steps:
                    for j in (0, 1):
                        mm(rzA[:], stat(j), wh_t[j][:, 0:512],
                           start=False, stop=False)
                        mm(C[:, 0:256], stat(j), wh_t[j][:, 1024:1280],
                           start=False, stop=False)
                        mm(rzB[:], stat(j), wh_t[j][:, 512:1024],
                           start=False, stop=False)
                        mm(C[:, 256:512], stat(j), wh_t[j][:, 1280:1536],
                           start=False, stop=False)
                for j in (0, 1):
                    mm(E[:], stat(j), wh_t[j][:, 1536:1540],
                       start=False, stop=False)

            def transp(j):
                nc.tensor.transpose(Tt[:, 32 * j: 32 * (j + 1)],
                                    h_bm[:, 128 * j: 128 * (j + 1)], id_t[:])

            def sig_half(half):
                # half 0 = A (features 0:256), 1 = B (256:512)
                src = rzA if half == 0 else rzB
                nc.scalar.activation(rz_sb[:, 512 * half: 512 * (half + 1)],
                                     src[:], AF.Sigmoid)

            def uw_half(half):
                r_sl = rz_sb[:, 512 * half: 512 * half + 256]
                u = wpool.tile([Bc, HH], f32, tag=f"u{half}")
                nc.vector.tensor_mul(u[:], r_sl,
                                     C[:, 256 * half: 256 * (half + 1)])
                w = wpool.tile([Bc, HH], f32, tag=f"w{half}")
                nc.vector.tensor_add(w[:], u[:],
                                     D[:, 256 * half: 256 * (half + 1)])
                return w

            def tanh_half(half, w):
                nc.scalar.activation(n_sb[:, 256 * half: 256 * (half + 1)],
                                     w[:], AF.Tanh)

            def hupd_half(half):
                z_sl = rz_sb[:, 512 * half + 256: 512 * half + 512]
                n_sl = n_sb[:, 256 * half: 256 * (half + 1)]
                h_sl = h_bm[:, 256 * half: 256 * (half + 1)]
                d1 = wpool.tile([Bc, HH], bf16, tag=f"d1{half}")
                nc.vector.tensor_sub(d1[:], h_sl, n_sl)
                d2 = wpool.tile([Bc, HH], bf16, tag=f"d2{half}")
                nc.vector.tensor_mul(d2[:], z_sl, d1[:])
                nc.vector.tensor_add(h_sl, n_sl, d2[:])

            # ---- iteration 0: k4(0) with immediate stop; gates; prep s=1
            E_cur = k4(0, first=True)
            sig_half(0)
            sig_half(1)
            wA0 = uw_half(0)
            tanh_half(0, wA0)
            hupd_half(0)
            wB0 = uw_half(1)
            tanh_half(1, wB0)
            hupd_half(1)
            E_next = k4(1)
            transp(0)
            transp(1)
            nc.vector.tensor_copy(h_tiles[:, 0:64], Tt[:, 0:64])
            early_mms(1, E_next)

            # ---- steady iterations
            for s in range(1, t_steps + 1):
                E_cur = E_next
                last = s == t_steps
                co = Bc * s
                # [PE] transposes j2/j3 of h_{s-1} half B
                transp(2)
                transp(3)
                # [DVE] copy them to SBUF stationaries
                nc.vector.tensor_copy(h_tiles[:, 64:128], Tt[:, 64:128])
                # [PE] scan2 F matmuls (dep: xtb from iter s-1)
                mm(F, xtb[0:2, :], wax_t[:], start=True, stop=False)
                mm(F, sv8[0:3, co: co + Bc], wav_t[:], start=False, stop=True)
                # [PE] tail matmuls: A group first (A chain feeds transp01)
                tail_mms(s, E_cur)
                # gate chain heads
                if not last:
                    sig_half(0)        # act
                    sig_half(1)        # act
                    wA = uw_half(0)    # DVE x2
                    tanh_half(0, wA)   # act
                    hupd_half(0)       # DVE x3 -> h'A
                # [act] scan2 relu x2 with row-sum accumulators
                hid = wpool.tile([Bc, HH], bf16, tag="hid")
                dtp = wpool.tile([Bc, 1], f32, tag="dtp")
                dtn = wpool.tile([Bc, 1], f32, tag="dtn")
                nc.scalar.activation(hid[:, 0:n_pos], misc[:, 0:n_pos],
                                     AF.Relu, accum_out=dtp[:])
                nc.scalar.activation(hid[:, n_pos:HH], misc[:, n_pos:HH],
                                     AF.Relu, accum_out=dtn[:])
                if not last:
                    wB = uw_half(1)    # DVE x2
                    # [PE] next step's gx (after all rz/C/D reads emitted)
                    E_next = k4(s + 1)
                    # [PE] transposes j0/j1 of h_s half A (waits h'A)
                    transp(0)
                    transp(1)
                    # [DVE] copy to stationaries
                    nc.vector.tensor_copy(h_tiles[:, 0:64], Tt[:, 0:64])
                    tanh_half(1, wB)   # act
                    # [PE] early matmuls for step s+1
                    early_mms(s + 1, E_next)
                # [DVE] dt = sum(pos relu) - sum(neg relu); [act] sigmoid
                dt_ = wpool.tile([Bc, 1], f32, tag="dt")
                nc.vector.tensor_sub(dt_[:], dtp[:], dtn[:])
                aw0 = wpool.tile([Bc, 1], f32, tag="aw0")
                nc.scalar.activation(aw0[:], dt_[:], AF.Sigmoid,
                                     bias=dbias_t[:])
                x2 = wpool.tile([Bc, 2], f32, tag="x2")
                xprev = x0_t[:] if s == 1 else xstg[:, 0:2]
                nc.vector.tensor_add(x2[:], E_cur[:, 2:4], xprev)
                # [DVE] B-half h update (tanhB done by now)
                if not last:
                    hupd_half(1)       # DVE x3 -> h'B
                # [DVE] x_{s-1} = aw0*E01 + (x_{s-2}+E23), fused
                nc.vector.scalar_tensor_tensor(
                    out=out_buf[:, 2 * (s - 1): 2 * s], in0=E_cur[:, 0:2],
                    scalar=aw0[:], in1=x2[:], op0=OP.mult, op1=OP.add)
                if not last:
                    # [DVE] x^T for next F matmul via staging tile
                    nc.vector.tensor_copy(xstg[:, 0:2],
                                          out_buf[:, 2 * (s - 1): 2 * s])
                    nc.vector.transpose(xtf[:], xstg[:])
                    nc.vector.tensor_copy(xtb[0:2, :], xtf[0:2, :])

            nc.sync.dma_start(out=d_out[:], in_=out_buf[:, 0:2 * T])

    nc.compile()
    return nc


# ------------------------------------------------------------------ interface

def kernel(X0, V, W_ih, W_hh, b_ih, b_hh, Wa1, ba1, Wa2, ba2, Wr, br,
           _trace=False, _tmpdir=None):
    from concourse.bass_utils import run_bass_kernel_spmd

    consts, n_pos = _prep_consts(W_ih, W_hh, b_ih, b_hh, Wa1, ba1, Wa2, ba2,
                                 Wr, br)
    key = (T, n_pos)
    if key not in _PROG_CACHE:
        _PROG_CACHE[key] = _build_program(T, n_pos)
    nc = _PROG_CACHE[key]
    in_maps = []
    for c in range(NCORES):
        core = _prep_core(c, X0, V)
        in_maps.append({**consts, **core})

    res = run_bass_kernel_spmd(nc, in_maps, list(range(NCORES)),
                               trace=_trace, tmpdir=_tmpdir)
    outs = [res.results[c]["out"].reshape(Bc, T, OUT) for c in range(NCORES)]
    out = np.concatenate(outs, axis=0).astype(np.float32)
    if _trace:
        return out, res
    return out
